# revision 47
# baseline (speedup 1.0000x reference)
"""AssignAttention (hard-routing slot attention) Trainium2 kernel, 8-core data-parallel.

Problem: B=16, N=64 groups, S=4096 tokens, C=768, H=8 heads, HD=96.
  q = query @ Wq.T; k = key @ Wk.T; v = key @ Wv.T (per-head split)
  logits = q @ k.T; hard-argmax over the 64 groups per token -> one-hot
  (softmax and the *SCALE factor are argmax-invariant, so both are skipped);
  attn = onehot / (count + 1); out = (attn @ v per head) @ Wo.T + bo

Sharding: data-parallel over batch B: 16 batches / 8 cores = 2 per core.
No collectives; the host concatenates per-core outputs.

Algorithm per core (validated vs fp32 reference: rel_l2 ~6e-3, the
residual being argmax flips on near-ties; measured ~495us on silicon,
down from ~595us for the fp16-x3 predecessor):
  - Logits REASSOCIATED: Y[c, (h,n)] = sum_d Wk[d(head h), c] q_proj[n, d]
    (tiny per batch); logits[s, (h,n)] = sum_c keyT[c, s] Y[c, (h,n)].
    The k-projection matmul disappears entirely.
  - Precision on the argmax path via a SCALED fp16 + fp8-DoubleRow split:
    all terms accumulate at 2^12 scale in ONE PSUM bank (argmax is
    scale-invariant, so the scale never needs removing):
      T1 = Kh16 @ (Yh16*2^12)                 6 fp16 matmuls
      T2+T3 = DR([fp8(Kh16), fp8(Kl*2^12)] @ [fp8(Yl*2^12), fp8(Yh16)])
                                              6 fp8 DoubleRow matmuls (K=256)
    (was 18 fp16 matmuls; a DR matmul costs the same ~216ns as a normal MM
    but contracts 2 fp8 planes). Residual logit rms err ~3e-5 (measured);
    the same scheme is applied to the q-projection (scale removed by the
    ACT copy out of PSUM). Hi/lo splits are exact: fp16/fp8 power-of-2
    scaling commutes with rounding in range.
  - argmax via row-max + (x >= max); counts via a ones-column in the
    group-sum rhs; renorm = per-partition reciprocal.
  - keyT via PE transpose-mode fp32 (3 blocks/PSUM bank, strided ACT copy);
    hi/lo/fp8 operand planes derived on DVE with PLAIN ops only
    (scalar_tensor_tensor / gpsimd elementwise / ACT-on-critical-path all
    measured 2-20x slower and stall the SWDGE DMA queue).
  - v-path REASSOCIATED: gs_raw[n,c] = onehot^T @ key (f16 rhs, head-pairs
    packed on PSUM partitions, counts as ones column), divided by count+1,
    then 64 group vectors projected through WvT. c split in two passes over
    S to fit PSUM (one-hots retained in SBUF, key cols re-streamed).
  - PE never waits on the argmax: group-sum matmuls for subtile i are
    emitted after subtile i+1's logits (per-engine queues execute in
    emission order, so this software-pipelines PE vs DVE).
  - Start/tail scheduling: key chunks 0-1 transposed/split BEFORE weight
    prep; Wv/Wo prep spread one block per chunk through the loop; batch 1's
    first key chunks prepped during batch 0 pass 2; the Wv projection of
    c[0:384] runs DURING pass 2 (its ga columns are final after pass 1).
  - Wo: single-pass fp16; bias via a K=1 fp32 outer-product matmul.
  - Engine split: PE matmuls/transposes; DVE splits/casts/argmax/divides;
    ACT PSUM->SBUF copies + off-critical weight/Y casts; SWDGE (gpsimd)
    bulk key DMA ONLY (gpsimd elementwise is catastrophically slow); HWDGE
    the rest.
"""

import sys

if "/opt/trn_rl_repo" not in sys.path:
    sys.path.insert(0, "/opt/trn_rl_repo")

import numpy as np

import concourse.bass as bass
import concourse.mybir as mybir
from concourse import bacc
import concourse.tile as tile
from concourse.masks import make_identity

f32 = mybir.dt.float32
f16 = mybir.dt.float16
f8 = mybir.dt.float8e4

C = 768
H = 8
HD = 96
NG = 64  # groups
CT = C // 128  # 6 c-tiles
S_CHUNK = 256
SC = 4096.0  # 2^12 split scale


def build_nc(b_sh=2, S=4096):
    nc = bacc.Bacc()

    query_d = nc.declare_dram_parameter("query", [b_sh, NG, C], f32, isOutput=False)
    key_d = nc.declare_dram_parameter("key_in", [b_sh, S, C], f32, isOutput=False)
    wq_d = nc.declare_dram_parameter("Wq", [C, C], f32, isOutput=False)
    wk_d = nc.declare_dram_parameter("Wk", [C, C], f32, isOutput=False)
    wv_d = nc.declare_dram_parameter("Wv", [C, C], f32, isOutput=False)
    wo_d = nc.declare_dram_parameter("Wo", [C, C], f32, isOutput=False)
    bo_d = nc.declare_dram_parameter("bo", [C], f32, isOutput=False)
    out_d = nc.declare_dram_parameter("out", [b_sh, NG, C], f32, isOutput=True)

    n_chunks = S // S_CHUNK
    n_sub = S_CHUNK // 128  # s-subtiles per chunk

    with tile.TileContext(nc) as tc:
        with (
            tc.tile_pool(name="wconst", bufs=1) as wconst,
            tc.tile_pool(name="qpool", bufs=1) as qpool,
            tc.tile_pool(name="ytmp", bufs=2) as ytmp,
            tc.tile_pool(name="ypool", bufs=2) as ypool,
            tc.tile_pool(name="ktmp", bufs=2) as ktmp,
            tc.tile_pool(name="kin", bufs=2) as kin,
            tc.tile_pool(name="keyT", bufs=2) as keyTp,
            tc.tile_pool(name="ohp", bufs=32) as ohp,
            tc.tile_pool(name="khip", bufs=3) as khip,
            tc.tile_pool(name="mxp", bufs=3) as mxp,
            tc.tile_pool(name="outp", bufs=1) as outp,
            tc.tile_pool(name="ps_a", bufs=2, space="PSUM") as ps_a,
            tc.tile_pool(name="ps_tr", bufs=2, space="PSUM") as ps_tr,
            tc.tile_pool(name="ps_g4", bufs=4, space="PSUM") as ps_g4,
        ):
            # ---- constants ----
            ident64_16 = wconst.tile([NG, NG], f16)
            make_identity(nc, ident64_16[:])
            ident64_32 = wconst.tile([NG, NG], f32)
            make_identity(nc, ident64_32[:])
            ident128_16 = wconst.tile([128, 128], f16)
            make_identity(nc, ident128_16[:])
            ident128_32 = wconst.tile([128, 128], f32)
            make_identity(nc, ident128_32[:])
            ones_row = wconst.tile([1, NG], f32)
            nc.vector.memset(ones_row[:], 1.0)
            bo_sb = wconst.tile([1, C], f32)
            nc.sync.dma_start(out=bo_sb[:], in_=bo_d[:].unsqueeze(0))

            def pe_transpose_psum(src, rows=128, u0=0, u1=CT):
                """PE-transpose f16 [rows,128] blocks src[:, 128u:128u+128]
                (u=u0..u1-1) into one batched PSUM tile; caller copies out."""
                ident = ident128_16 if rows == 128 else ident64_16
                trp = ps_tr.tile([128, u1 - u0, rows], f16, tag="pstr")
                for u in range(u0, u1):
                    nc.tensor.matmul(
                        trp[:, u - u0, :],
                        src[0:rows, 128 * u : 128 * u + 128],
                        ident[:],
                        is_transpose=True,
                        start=(u == u0),
                        stop=(u == u1 - 1),
                    )
                return trp

            def pe_transpose_blocks(src, dst, t, rows=128, u0=0, u1=CT):
                """PE-transpose f16 blocks + single strided ACT copy into
                dst[:, u0:u1, 128t:128t+rows]."""
                trp = pe_transpose_psum(src, rows=rows, u0=u0, u1=u1)
                nc.scalar.copy(out=dst[:, u0:u1, 128 * t : 128 * t + rows], in_=trp[:])

            # ---- per-chunk key prep: DMA, transpose, split, fp8 planes ----
            key_tiles = {}  # (b, chunk) -> (kTh, k8, khi)

            def phase_k(b, chunk):
                s0 = chunk * S_CHUNK
                knat = kin.tile([128, n_sub, C], f32, tag="knat")
                nc.gpsimd.dma_start(
                    out=knat[:],
                    in_=key_d[b, s0 : s0 + S_CHUNK, :].rearrange(
                        "(i p) c -> p i c", p=128
                    ),
                )
                keyT = keyTp.tile([128, CT, S_CHUNK], f32, tag="keyT")
                for i in range(n_sub):
                    for g in range(2):
                        trp = ps_tr.tile([128, 3, 128], f32, tag="pstr")
                        for j in range(3):
                            nc.tensor.matmul(
                                trp[:, j, :],
                                knat[:, i, 128 * (3 * g + j) : 128 * (3 * g + j) + 128],
                                ident128_32[:],
                                is_transpose=True,
                                start=(j == 0),
                                stop=(j == 2),
                            )
                        nc.scalar.copy(
                            out=keyT[:, 3 * g : 3 * g + 3, 128 * i : 128 * i + 128],
                            in_=trp[:],
                        )
                # kTh = fp16(keyT); fp8 planes k8 = [fp8(kh), fp8(kl*2^12)]
                # (plain DVE ops only -- fused/gpsimd/ACT variants measured
                # 2-20x slower or stall the critical path).
                kTh = keyTp.tile([128, CT, S_CHUNK], f16, tag="kTh")
                k8 = keyTp.tile([128, CT, 2, S_CHUNK], f8, tag="k8")
                for i in range(n_sub):
                    isl = slice(128 * i, 128 * i + 128)
                    nc.vector.tensor_copy(kTh[:, :, isl], keyT[:, :, isl])
                    ktl = ktmp.tile([128, CT, 128], f16, tag="ktl")
                    nc.vector.tensor_tensor(
                        out=ktl[:], in0=keyT[:, :, isl],
                        in1=kTh[:, :, isl], op=mybir.AluOpType.subtract,
                    )
                    nc.vector.tensor_copy(k8[:, :, 0, isl], kTh[:, :, isl])
                    nc.vector.tensor_scalar(
                        out=k8[:, :, 1, isl], in0=ktl[:],
                        scalar1=SC, scalar2=None, op0=mybir.AluOpType.mult,
                    )
                # natural-layout f16 key, first c-half + ones column (counts)
                khi = khip.tile([128, n_sub, 509], f16, tag="khi")
                nc.vector.tensor_copy(khi[:, :, 0:508], knat[:, :, 0:508])
                nc.vector.memset(khi[:, :, 508], 1.0)
                key_tiles[(b, chunk)] = (kTh, k8, khi)

            # ---- prefetch + fully prep key chunks 0-1 of batch 0 BEFORE
            # weight prep: fills the PE/DVE while weight DMAs stream in.
            phase_k(0, 0)
            phase_k(0, 1)

            # ---- weight prep: Wq transposed c-major fp16 hi + fp8 DR planes
            # wq8 = [fp8(wq_lo*2^12), fp8(wq_hi)] (lo plane pairs with q-hi);
            # Wk natural fp32.
            CP = 128 * H  # d-padded width for Wq/Wk (head h at 128h..128h+96)
            wqT_h = wconst.tile([128, CT, CP], f16)
            wq8 = wconst.tile([128, CT, 2, CP], f8)
            wk_nat = wconst.tile([128, H, C], f32)
            wvT_h = wconst.tile([128, CT, C], f16)
            woT_h = wconst.tile([128, CT, C], f16)

            wtmp_ctx = tc.tile_pool(name="wtmp", bufs=2)
            wtmp = wtmp_ctx.__enter__()
            for hd in range(H):
                wnat = wtmp.tile([128, C], f32, tag="wnat")
                nc.vector.memset(wnat[96:128, :], 0.0)
                eng = nc.sync if hd % 2 == 0 else nc.scalar
                eng.dma_start(
                    out=wnat[0:HD, :], in_=wq_d[HD * hd : HD * hd + HD, :]
                )
                whi = wtmp.tile([128, C], f16, tag="whi")
                nc.scalar.copy(out=whi[:], in_=wnat[:])
                trp = pe_transpose_psum(whi[:])
                nc.scalar.copy(out=wqT_h[:, :, 128 * hd : 128 * hd + 128], in_=trp[:])
                nc.vector.tensor_copy(wq8[:, :, 1, 128 * hd : 128 * hd + 128], trp[:])
                wlo = wtmp.tile([128, C], f16, tag="wlo")
                nc.vector.tensor_tensor(
                    out=wlo[:], in0=wnat[:], in1=whi[:], op=mybir.AluOpType.subtract
                )
                trp2 = pe_transpose_psum(wlo[:])
                nc.vector.tensor_scalar(
                    out=wq8[:, :, 0, 128 * hd : 128 * hd + 128], in0=trp2[:],
                    scalar1=SC, scalar2=None, op0=mybir.AluOpType.mult,
                )
            nc.vector.memset(wk_nat[96:128, :, :], 0.0)
            for hd in range(H):
                eng = nc.sync if hd % 2 == 0 else nc.scalar
                eng.dma_start(
                    out=wk_nat[0:HD, hd, :], in_=wk_d[HD * hd : HD * hd + HD, :]
                )

            # Wv/Wo prep split into per-block steps, spread across the chunk
            # loop (one DMA + one transpose per chunk, software-pipelined).
            wvwo_blocks = [(wv_d, wvT_h, t) for t in range(CT)] + [
                (wo_d, woT_h, t) for t in range(CT)
            ]
            wvwo_staged = []
            wv_step = [0]

            def prep_wv_wo_step(step):
                # stage the DMA for block `step`, transpose block `step-1`
                if step < len(wvwo_blocks):
                    w_dram, dst_h, t = wvwo_blocks[step]
                    wnat = wtmp.tile([128, C], f32, tag="wnat")
                    eng = nc.sync if step % 2 == 0 else nc.scalar
                    eng.dma_start(
                        out=wnat[:], in_=w_dram[128 * t : 128 * t + 128, :]
                    )
                    whi = wtmp.tile([128, C], f16, tag="whi")
                    nc.vector.tensor_copy(whi[:], wnat[:])
                    wvwo_staged.append((whi, dst_h, t))
                if step > 0 and step - 1 < len(wvwo_blocks):
                    whi, dst_h, t = wvwo_staged[step - 1]
                    pe_transpose_blocks(whi[:], dst_h[:], t)
                if step - 1 == len(wvwo_blocks) - 1:
                    wtmp_ctx.__exit__(None, None, None)

            qy_tiles = {}

            def prep_qy(b):
                # ---- Q path ----
                q_nat = qpool.tile([NG, C], f32, tag="qnat")
                nc.sync.dma_start(out=q_nat[:], in_=query_d[b])
                qh_nat = qpool.tile([NG, C], f16, tag="qhnat")
                ql_nat = qpool.tile([NG, C], f16, tag="qlnat")
                nc.vector.tensor_copy(qh_nat[:], q_nat[:])
                nc.vector.tensor_tensor(
                    out=ql_nat[:], in0=q_nat[:], in1=qh_nat[:], op=mybir.AluOpType.subtract
                )
                # queryT (c-major) fp16 halves via PE transpose ([64,128] blocks)
                qTq_h = qpool.tile([128, CT, NG], f16, tag="qTqh")
                qTq_l = qpool.tile([128, CT, NG], f16, tag="qTql")
                for qsrc, dst in ((qh_nat, qTq_h), (ql_nat, qTq_l)):
                    pe_transpose_blocks(qsrc, dst[:].unsqueeze(3).rearrange("p u n o -> p u (n o)"), 0, rows=NG)
                # DR operands for q-proj: qs16 = qh*2^12, q8 = [fp8(qh), fp8(ql*2^12)]
                qs16 = qpool.tile([128, CT, NG], f16, tag="qs16")
                q8 = qpool.tile([128, CT, 2, NG], f8, tag="q8")
                nc.vector.tensor_scalar(
                    out=qs16[:], in0=qTq_h[:], scalar1=SC, scalar2=None,
                    op0=mybir.AluOpType.mult,
                )
                nc.vector.tensor_copy(q8[:, :, 0, :], qTq_h[:])
                nc.vector.tensor_scalar(
                    out=q8[:, :, 1, :], in0=qTq_l[:], scalar1=SC, scalar2=None,
                    op0=mybir.AluOpType.mult,
                )
                # q projection (natural layout, M=64), d-padded: q_pad [64, 1024]
                # 6 fp16 + 6 DR matmuls at 2^12 scale, unscaled on the ACT copy
                q_sb = qpool.tile([NG, CP], f32, tag="qsb")
                for half in range(2):
                    nsl = slice(512 * half, 512 * half + 512)
                    nsl8 = slice(512 * half, 512 * half + 512)
                    qp = ps_a.tile([NG, 512], f32, tag="psa")
                    for u in range(CT):
                        nc.tensor.matmul(
                            qp[:],
                            qs16[:, u, :],
                            wqT_h[:, u, nsl],
                            start=(u == 0),
                            stop=False,
                        )
                    for u in range(CT):
                        nc.tensor.matmul(
                            qp[:],
                            q8[:, u, :, :],
                            wq8[:, u, :, nsl8],
                            start=False,
                            stop=(u == CT - 1),
                            perf_mode=mybir.MatmulPerfMode.DoubleRow,
                        )
                    nc.scalar.mul(q_sb[:, nsl], qp[:], 1.0 / SC)
                # qT (padded d-major, per head) fp32 via PE transpose
                qT = qpool.tile([128, H, NG], f32, tag="qT")
                for hd in range(H):
                    trq2 = ps_a.tile([128, NG], f32, tag="psa")
                    nc.tensor.matmul(
                        trq2[:],
                        q_sb[:, 128 * hd : 128 * hd + 128],
                        ident64_32[:],
                        is_transpose=True,
                        start=True,
                        stop=True,
                    )
                    nc.scalar.copy(out=qT[:, hd, :], in_=trq2[:])
                # Y_all[c, 64h+n] = sum_d Wk[d(head h), c] * q[n, d], fp32;
                # split into T1 rhs Yh16s = fp16(Y)*2^12 (exact p2 scale) and
                # fp8 planes y8 = [fp8(Yl*2^12), fp8(Yh16)].
                Yh16s = ypool.tile([128, CT, 8 * NG], f16, tag="Yh16s")
                y8 = ypool.tile([128, CT, 2, 8 * NG], f8, tag="y8")
                for u_c in range(CT):
                    yp = ps_a.tile([128, 8 * NG], f32, tag="psa")
                    csl = slice(128 * u_c, 128 * u_c + 128)
                    for hd in range(H):
                        nc.tensor.matmul(
                            yp[:, NG * hd : NG * hd + NG],
                            wk_nat[:, hd, csl],
                            qT[:, hd, :],
                            start=(hd == 0),
                            stop=(hd == H - 1),
                        )
                    yh = ytmp.tile([128, 8 * NG], f16, tag="yh")
                    nc.vector.tensor_copy(yh[:], yp[:])
                    nc.scalar.mul(Yh16s[:, u_c, :], yh[:], SC)
                    ylt = ytmp.tile([128, 8 * NG], f16, tag="ylt")
                    nc.vector.tensor_tensor(
                        out=ylt[:], in0=yp[:], in1=yh[:], op=mybir.AluOpType.subtract
                    )
                    nc.vector.tensor_scalar(
                        out=y8[:, u_c, 0, :], in0=ylt[:],
                        scalar1=SC, scalar2=None, op0=mybir.AluOpType.mult,
                    )
                    nc.scalar.copy(out=y8[:, u_c, 1, :], in_=yh[:])
                qy_tiles[b] = (Yh16s, y8)

            for b in range(b_sh):
                if b not in qy_tiles:
                    prep_qy(b)
                Yh16s, y8 = qy_tiles.pop(b)

                # ---- raw-key group-sum accumulators (head-pair packed):
                # gsr[j][n(2 heads), c-half+count] = sum_s onehot[s, n] key[s, c]
                gsr = [ps_g4.tile([128, 509], f32, tag="g4", name=f"gsr{_j}") for _j in range(4)]
                oh_tiles = []
                gs_pending = None

                def emit_gs1(oh_t, khi_t, i_t, first_t, last_t):
                    for j in range(4):
                        nc.tensor.matmul(
                            gsr[j][:],
                            oh_t[:, 128 * j : 128 * j + 128],
                            khi_t[:, i_t, :],
                            start=first_t,
                            stop=last_t,
                        )

                for chunk in range(n_chunks):
                    if (b, chunk) not in key_tiles:
                        phase_k(b, chunk)
                    if b == 0 and chunk >= 3 and wv_step[0] <= len(wvwo_blocks):
                        prep_wv_wo_step(wv_step[0])
                        wv_step[0] += 1
                    if b == 0 and b_sh > 1 and chunk == 12:
                        prep_qy(1)
                    kTh, k8, khi = key_tiles.pop((b, chunk))

                    for i in range(n_sub):
                        ssl = slice(128 * i, 128 * i + 128)
                        # logits*2^12 for all 8 heads: 6 fp16 + 6 fp8-DR MMs
                        # into one PSUM accumulation group.
                        lg = ps_a.tile([128, 8 * NG], f32, tag="psa")
                        for u_c in range(CT):
                            nc.tensor.matmul(
                                lg[:],
                                kTh[:, u_c, ssl],
                                Yh16s[:, u_c, :],
                                start=(u_c == 0),
                                stop=False,
                            )
                        for u_c in range(CT):
                            nc.tensor.matmul(
                                lg[:],
                                k8[:, u_c, :, ssl],
                                y8[:, u_c, :, :],
                                start=False,
                                stop=(u_c == CT - 1),
                                perf_mode=mybir.MatmulPerfMode.DoubleRow,
                            )
                        # argmax -> one-hot via (x >= rowmax), fp16
                        mx = mxp.tile([128, H], f32, tag="mx")
                        lg3 = lg[:].rearrange("p (h n) -> p h n", h=H)
                        nc.vector.tensor_reduce(
                            out=mx[:],
                            in_=lg3,
                            axis=mybir.AxisListType.X,
                            op=mybir.AluOpType.max,
                        )
                        # one-hot in fp8 (0/1 exact; fp8-lhsT x f16-rhs
                        # matmul verified exact on HW) -- halves oh SBUF
                        oh = ohp.tile([128, H * NG], f8, tag="oh")
                        nc.vector.tensor_tensor(
                            out=oh[:].rearrange("p (h n) -> p h n", h=H),
                            in0=lg3,
                            in1=mx[:].unsqueeze(2).to_broadcast((128, H, NG)),
                            op=mybir.AluOpType.is_ge,
                        )

                        oh_tiles.append(oh)
                        # pass 1 gs_raw is emitted one subtile LATE (after the
                        # next subtile's logits) so the PE never waits on the
                        # DVE argmax: gs(i) lands in the PE queue after
                        # logits(i+1).
                        if gs_pending is not None:
                            emit_gs1(*gs_pending)
                        gs_pending = (oh, khi, i, chunk == 0 and i == 0, False)
                if gs_pending is not None:
                    # flush the final subtile's gs matmuls (accum group stop)
                    oh_p, khi_p, i_p, first_p, _ = gs_pending
                    emit_gs1(oh_p, khi_p, i_p, first_p, True)
                    gs_pending = None

                while b == 0 and wv_step[0] <= len(wvwo_blocks):
                    prep_wv_wo_step(wv_step[0])
                    wv_step[0] += 1

                # ---- recip of counts, divide pass-1 halves into ga ----
                cnts = outp.tile([128, 4], f32, tag="cnts")
                recs = outp.tile([128, 4], f32, tag="recs")
                ga = outp.tile([128, 4, C], f16, tag="ga")
                for j in range(4):
                    nc.vector.tensor_scalar(
                        out=cnts[:, j : j + 1], in0=gsr[j][:, 508:509],
                        scalar1=1.0, scalar2=None, op0=mybir.AluOpType.add,
                    )
                    nc.vector.reciprocal(recs[:, j : j + 1], cnts[:, j : j + 1])
                    nc.vector.tensor_scalar(
                        out=ga[:, j, 0:508], in0=gsr[j][:, 0:508],
                        scalar1=recs[:, j : j + 1], scalar2=None,
                        op0=mybir.AluOpType.mult,
                    )

                # ---- vproj part A (c 0:384) overlaps pass 2: transpose the
                # ready ga columns and accumulate the first 3 c-chunks of the
                # Wv projection while pass 2 streams.
                gaT = outp.tile([128, CT, 4, 128], f16, tag="gaT")
                for j in range(4):
                    pe_transpose_blocks(ga[:, j, :], gaT[:, :, j, :], 0, u0=0, u1=3)
                attn16 = outp.tile([NG, C], f16, tag="attn16")
                for h in range(H):
                    pa = ps_a.tile([NG, HD], f32, tag="psa")
                    for u_c in range(3):
                        nc.tensor.matmul(
                            pa[:],
                            gaT[:, u_c, h // 2, 64 * (h % 2) : 64 * (h % 2) + 64],
                            wvT_h[:, u_c, HD * h : HD * h + HD],
                            start=(u_c == 0),
                            stop=(u_c == 2),
                        )
                    nc.scalar.copy(out=attn16[:, HD * h : HD * h + HD], in_=pa[:])

                # ---- pass 2: re-stream key, gs_raw over c[508:768] ----
                gsr2 = [ps_g4.tile([128, 260], f32, tag="g4", name=f"gsr2_{_j}") for _j in range(4)]
                for chunk in range(n_chunks):
                    s0 = chunk * S_CHUNK
                    knat2 = kin.tile([128, n_sub, C], f32, tag="knat")
                    nc.gpsimd.dma_start(
                        out=knat2[:, :, 0:260],
                        in_=key_d[b, s0 : s0 + S_CHUNK, 508:768].rearrange(
                            "(i p) c -> p i c", p=128
                        ),
                    )
                    khi2 = khip.tile([128, n_sub, 509], f16, tag="khi")
                    nc.vector.tensor_copy(khi2[:, :, 0:260], knat2[:, :, 0:260])
                    for i in range(n_sub):
                        last = chunk == n_chunks - 1 and i == n_sub - 1
                        first = chunk == 0 and i == 0
                        oh_t = oh_tiles[chunk * n_sub + i]
                        for j in range(4):
                            nc.tensor.matmul(
                                gsr2[j][:, 0:260],
                                oh_t[:, 128 * j : 128 * j + 128],
                                khi2[:, i, 0:260],
                                start=first,
                                stop=last,
                            )
                    # overlap batch 1's first key preps with batch 0 pass 2
                    if b == 0 and b_sh > 1 and chunk == n_chunks - 2:
                        phase_k(1, 0)
                    if b == 0 and b_sh > 1 and chunk == n_chunks - 1:
                        phase_k(1, 1)
                for j in range(4):
                    nc.vector.tensor_scalar(
                        out=ga[:, j, 508:768], in0=gsr2[j][:, 0:260],
                        scalar1=recs[:, j : j + 1], scalar2=None,
                        op0=mybir.AluOpType.mult,
                    )

                # ---- vproj part B (c 384:768): transpose remaining ga
                # columns, accumulate, and DVE-add into attn16 ----
                for j in range(4):
                    pe_transpose_blocks(ga[:, j, :], gaT[:, :, j, :], 0, u0=3, u1=CT)
                for h in range(H):
                    pa = ps_a.tile([NG, HD], f32, tag="psa")
                    for u_c in range(3, CT):
                        nc.tensor.matmul(
                            pa[:],
                            gaT[:, u_c, h // 2, 64 * (h % 2) : 64 * (h % 2) + 64],
                            wvT_h[:, u_c, HD * h : HD * h + HD],
                            start=(u_c == 3),
                            stop=(u_c == CT - 1),
                        )
                    nc.vector.tensor_tensor(
                        out=attn16[:, HD * h : HD * h + HD],
                        in0=pa[:],
                        in1=attn16[:, HD * h : HD * h + HD],
                        op=mybir.AluOpType.add,
                    )
                attnT = outp.tile([128, CT, NG], f16, tag="attnT")
                pe_transpose_blocks(attn16, attnT[:].unsqueeze(3).rearrange("p u n o -> p u (n o)"), 0, rows=NG)

                out_sb = outp.tile([NG, C], f32, tag="outsb")
                for half in range(2):
                    nsl = slice(384 * half, 384 * half + 384)
                    op = ps_a.tile([NG, 384], f32, tag="psa")
                    for u_c in range(CT):
                        nc.tensor.matmul(
                            op[:],
                            attnT[:, u_c, :],
                            woT_h[:, u_c, nsl],
                            start=(u_c == 0),
                            stop=False,
                        )
                    nc.tensor.matmul(
                        op[:], ones_row[:], bo_sb[:, nsl], start=False, stop=True
                    )
                    nc.scalar.copy(out=out_sb[:, nsl], in_=op[:])
                nc.gpsimd.dma_start(out=out_d[b], in_=out_sb[:])

    nc.finalize()
    return nc


_NC_CACHE = {}


def _get_nc(b_sh, S):
    key = (b_sh, S)
    if key not in _NC_CACHE:
        _NC_CACHE[key] = build_nc(b_sh, S)
    return _NC_CACHE[key]


def kernel(query, key_in, Wq, Wk, Wv, Wo, bo):
    from concourse.bass_utils import run_bass_kernel_spmd

    query = np.ascontiguousarray(np.asarray(query, dtype=np.float32))
    key_in = np.ascontiguousarray(np.asarray(key_in, dtype=np.float32))
    Wq = np.ascontiguousarray(np.asarray(Wq, dtype=np.float32))
    Wk = np.ascontiguousarray(np.asarray(Wk, dtype=np.float32))
    Wv = np.ascontiguousarray(np.asarray(Wv, dtype=np.float32))
    Wo = np.ascontiguousarray(np.asarray(Wo, dtype=np.float32))
    bo = np.ascontiguousarray(np.asarray(bo, dtype=np.float32))

    B, _, _ = query.shape
    S = key_in.shape[1]
    n_cores = 8
    b_sh = B // n_cores
    nc = _get_nc(b_sh, S)

    in_maps = []
    for i in range(n_cores):
        bs = slice(i * b_sh, (i + 1) * b_sh)
        in_maps.append(
            {
                "query": np.ascontiguousarray(query[bs]),
                "key_in": np.ascontiguousarray(key_in[bs]),
                "Wq": Wq,
                "Wk": Wk,
                "Wv": Wv,
                "Wo": Wo,
                "bo": bo,
            }
        )
    res = run_bass_kernel_spmd(nc, in_maps, core_ids=list(range(n_cores)))
    out = np.concatenate([res.results[i]["out"] for i in range(n_cores)], axis=0)
    return out.astype(np.float32)


if __name__ == "__main__":
    nc = build_nc(1, 512)
    print("built ok")


# revision 48
# speedup vs baseline: 1.0147x; 1.0147x over previous
"""AssignAttention (hard-routing slot attention) Trainium2 kernel, 8-core data-parallel.

Problem: B=16, N=64 groups, S=4096 tokens, C=768, H=8 heads, HD=96.
  q = query @ Wq.T; k = key @ Wk.T; v = key @ Wv.T (per-head split)
  logits = q @ k.T; hard-argmax over the 64 groups per token -> one-hot
  (softmax and the *SCALE factor are argmax-invariant, so both are skipped);
  attn = onehot / (count + 1); out = (attn @ v per head) @ Wo.T + bo

Sharding: data-parallel over batch B: 16 batches / 8 cores = 2 per core.
No collectives; the host concatenates per-core outputs.

Algorithm per core (validated vs fp32 reference: rel_l2 ~6e-3, the
residual being argmax flips on near-ties; measured ~495us on silicon,
down from ~595us for the fp16-x3 predecessor):
  - Logits REASSOCIATED: Y[c, (h,n)] = sum_d Wk[d(head h), c] q_proj[n, d]
    (tiny per batch); logits[s, (h,n)] = sum_c keyT[c, s] Y[c, (h,n)].
    The k-projection matmul disappears entirely.
  - Precision on the argmax path via a SCALED fp16 + fp8-DoubleRow split:
    all terms accumulate at 2^12 scale in ONE PSUM bank (argmax is
    scale-invariant, so the scale never needs removing):
      T1 = Kh16 @ (Yh16*2^12)                 6 fp16 matmuls
      T2+T3 = DR([fp8(Kh16), fp8(Kl*2^12)] @ [fp8(Yl*2^12), fp8(Yh16)])
                                              6 fp8 DoubleRow matmuls (K=256)
    (was 18 fp16 matmuls; a DR matmul costs the same ~216ns as a normal MM
    but contracts 2 fp8 planes). Residual logit rms err ~3e-5 (measured);
    the same scheme is applied to the q-projection (scale removed by the
    ACT copy out of PSUM). Hi/lo splits are exact: fp16/fp8 power-of-2
    scaling commutes with rounding in range.
  - argmax via row-max + (x >= max); counts via a ones-column in the
    group-sum rhs; renorm = per-partition reciprocal.
  - keyT via PE transpose-mode fp32 (3 blocks/PSUM bank, strided ACT copy);
    hi/lo/fp8 operand planes derived on DVE with PLAIN ops only
    (scalar_tensor_tensor / gpsimd elementwise / ACT-on-critical-path all
    measured 2-20x slower and stall the SWDGE DMA queue).
  - v-path REASSOCIATED: gs_raw[n,c] = onehot^T @ key (f16 rhs, head-pairs
    packed on PSUM partitions, counts as ones column), divided by count+1,
    then 64 group vectors projected through WvT. c split in two passes over
    S to fit PSUM (one-hots retained in SBUF, key cols re-streamed).
  - PE never waits on the argmax: group-sum matmuls for subtile i are
    emitted after subtile i+1's logits (per-engine queues execute in
    emission order, so this software-pipelines PE vs DVE).
  - Start/tail scheduling: key chunks 0-1 transposed/split BEFORE weight
    prep; Wv/Wo prep spread one block per chunk through the loop; batch 1's
    first key chunks prepped during batch 0 pass 2; the Wv projection of
    c[0:384] runs DURING pass 2 (its ga columns are final after pass 1).
  - Wo: single-pass fp16; bias via a K=1 fp32 outer-product matmul.
  - Engine split: PE matmuls/transposes; DVE splits/casts/argmax/divides;
    ACT PSUM->SBUF copies + off-critical weight/Y casts; SWDGE (gpsimd)
    bulk key DMA ONLY (gpsimd elementwise is catastrophically slow); HWDGE
    the rest.
"""

import sys

if "/opt/trn_rl_repo" not in sys.path:
    sys.path.insert(0, "/opt/trn_rl_repo")

import numpy as np

import concourse.bass as bass
import concourse.mybir as mybir
from concourse import bacc
import concourse.tile as tile
from concourse.masks import make_identity

f32 = mybir.dt.float32
f16 = mybir.dt.float16
f8 = mybir.dt.float8e4

C = 768
H = 8
HD = 96
NG = 64  # groups
CT = C // 128  # 6 c-tiles
S_CHUNK = 256
SC = 4096.0  # 2^12 split scale


def build_nc(b_sh=2, S=4096):
    nc = bacc.Bacc()

    query_d = nc.declare_dram_parameter("query", [b_sh, NG, C], f32, isOutput=False)
    key_d = nc.declare_dram_parameter("key_in", [b_sh, S, C], f32, isOutput=False)
    wq_d = nc.declare_dram_parameter("Wq", [C, C], f32, isOutput=False)
    wk_d = nc.declare_dram_parameter("Wk", [C, C], f32, isOutput=False)
    wv_d = nc.declare_dram_parameter("Wv", [C, C], f32, isOutput=False)
    wo_d = nc.declare_dram_parameter("Wo", [C, C], f32, isOutput=False)
    bo_d = nc.declare_dram_parameter("bo", [C], f32, isOutput=False)
    out_d = nc.declare_dram_parameter("out", [b_sh, NG, C], f32, isOutput=True)

    n_chunks = S // S_CHUNK
    n_sub = S_CHUNK // 128  # s-subtiles per chunk

    with tile.TileContext(nc) as tc:
        with (
            tc.tile_pool(name="wconst", bufs=1) as wconst,
            tc.tile_pool(name="qpool", bufs=1) as qpool,
            tc.tile_pool(name="ytmp", bufs=2) as ytmp,
            tc.tile_pool(name="ypool", bufs=2) as ypool,
            tc.tile_pool(name="ktmp", bufs=2) as ktmp,
            tc.tile_pool(name="kin", bufs=2) as kin,
            tc.tile_pool(name="keyT", bufs=2) as keyTp,
            tc.tile_pool(name="ohp", bufs=32) as ohp,
            tc.tile_pool(name="khip", bufs=3) as khip,
            tc.tile_pool(name="mxp", bufs=3) as mxp,
            tc.tile_pool(name="outp", bufs=1) as outp,
            tc.tile_pool(name="ps_a", bufs=2, space="PSUM") as ps_a,
            tc.tile_pool(name="ps_tr", bufs=2, space="PSUM") as ps_tr,
            tc.tile_pool(name="ps_g4", bufs=4, space="PSUM") as ps_g4,
        ):
            # ---- constants ----
            ident64_16 = wconst.tile([NG, NG], f16)
            make_identity(nc, ident64_16[:])
            ident64_32 = wconst.tile([NG, NG], f32)
            make_identity(nc, ident64_32[:])
            ident128_16 = wconst.tile([128, 128], f16)
            make_identity(nc, ident128_16[:])
            ident128_32 = wconst.tile([128, 128], f32)
            make_identity(nc, ident128_32[:])
            ones_row = wconst.tile([1, NG], f32)
            nc.vector.memset(ones_row[:], 1.0)
            bo_sb = wconst.tile([1, C], f32)
            nc.sync.dma_start(out=bo_sb[:], in_=bo_d[:].unsqueeze(0))

            def pe_transpose_psum(src, rows=128, u0=0, u1=CT):
                """PE-transpose f16 [rows,128] blocks src[:, 128u:128u+128]
                (u=u0..u1-1) into one batched PSUM tile; caller copies out."""
                ident = ident128_16 if rows == 128 else ident64_16
                trp = ps_tr.tile([128, u1 - u0, rows], f16, tag="pstr")
                for u in range(u0, u1):
                    nc.tensor.matmul(
                        trp[:, u - u0, :],
                        src[0:rows, 128 * u : 128 * u + 128],
                        ident[:],
                        is_transpose=True,
                        start=(u == u0),
                        stop=(u == u1 - 1),
                    )
                return trp

            def pe_transpose_blocks(src, dst, t, rows=128, u0=0, u1=CT):
                """PE-transpose f16 blocks + single strided ACT copy into
                dst[:, u0:u1, 128t:128t+rows]."""
                trp = pe_transpose_psum(src, rows=rows, u0=u0, u1=u1)
                nc.scalar.copy(out=dst[:, u0:u1, 128 * t : 128 * t + rows], in_=trp[:])

            # ---- per-chunk key prep: DMA, transpose, split, fp8 planes ----
            key_tiles = {}  # (b, chunk) -> (kTh, k8, khi)

            def phase_k(b, chunk):
                s0 = chunk * S_CHUNK
                knat = kin.tile([128, n_sub, C], f32, tag="knat")
                nc.gpsimd.dma_start(
                    out=knat[:],
                    in_=key_d[b, s0 : s0 + S_CHUNK, :].rearrange(
                        "(i p) c -> p i c", p=128
                    ),
                )
                keyT = keyTp.tile([128, CT, S_CHUNK], f32, tag="keyT")
                for i in range(n_sub):
                    for g in range(2):
                        trp = ps_tr.tile([128, 3, 128], f32, tag="pstr")
                        for j in range(3):
                            nc.tensor.matmul(
                                trp[:, j, :],
                                knat[:, i, 128 * (3 * g + j) : 128 * (3 * g + j) + 128],
                                ident128_32[:],
                                is_transpose=True,
                                start=(j == 0),
                                stop=(j == 2),
                            )
                        nc.scalar.copy(
                            out=keyT[:, 3 * g : 3 * g + 3, 128 * i : 128 * i + 128],
                            in_=trp[:],
                        )
                # kTh = fp16(keyT); fp8 planes k8 = [fp8(kh), fp8(kl*2^12)]
                # (plain DVE ops only -- fused/gpsimd/ACT variants measured
                # 2-20x slower or stall the critical path).
                kTh = keyTp.tile([128, CT, S_CHUNK], f16, tag="kTh")
                k8 = keyTp.tile([128, CT, 2, S_CHUNK], f8, tag="k8")
                for i in range(n_sub):
                    isl = slice(128 * i, 128 * i + 128)
                    nc.vector.tensor_copy(kTh[:, :, isl], keyT[:, :, isl])
                    ktl = ktmp.tile([128, CT, 128], f16, tag="ktl")
                    nc.vector.tensor_tensor(
                        out=ktl[:], in0=keyT[:, :, isl],
                        in1=kTh[:, :, isl], op=mybir.AluOpType.subtract,
                    )
                    nc.vector.tensor_copy(k8[:, :, 0, isl], kTh[:, :, isl])
                    nc.vector.tensor_scalar(
                        out=k8[:, :, 1, isl], in0=ktl[:],
                        scalar1=SC, scalar2=None, op0=mybir.AluOpType.mult,
                    )
                # natural-layout f16 key, first c-half + ones column (counts)
                khi = khip.tile([128, n_sub, 509], f16, tag="khi")
                nc.vector.tensor_copy(khi[:, :, 0:508], knat[:, :, 0:508])
                nc.vector.memset(khi[:, :, 508], 1.0)
                key_tiles[(b, chunk)] = (kTh, k8, khi)

            # ---- prefetch + fully prep key chunks 0-1 of batch 0 BEFORE
            # weight prep: fills the PE/DVE while weight DMAs stream in.
            phase_k(0, 0)
            phase_k(0, 1)

            # ---- weight prep: Wq transposed c-major fp16 hi + fp8 DR planes
            # wq8 = [fp8(wq_lo*2^12), fp8(wq_hi)] (lo plane pairs with q-hi);
            # Wk natural fp32.
            CP = 128 * H  # d-padded width for Wq/Wk (head h at 128h..128h+96)
            wqT_h = wconst.tile([128, CT, CP], f16)
            wq8 = wconst.tile([128, CT, 2, CP], f8)
            wk_nat = wconst.tile([128, H, C], f32)
            wvT_h = wconst.tile([128, CT, C], f16)
            woT_h = wconst.tile([128, CT, C], f16)

            wtmp_ctx = tc.tile_pool(name="wtmp", bufs=2)
            wtmp = wtmp_ctx.__enter__()
            for hd in range(H):
                wnat = wtmp.tile([128, C], f32, tag="wnat")
                nc.vector.memset(wnat[96:128, :], 0.0)
                eng = nc.sync if hd % 2 == 0 else nc.scalar
                eng.dma_start(
                    out=wnat[0:HD, :], in_=wq_d[HD * hd : HD * hd + HD, :]
                )
                whi = wtmp.tile([128, C], f16, tag="whi")
                nc.scalar.copy(out=whi[:], in_=wnat[:])
                trp = pe_transpose_psum(whi[:])
                nc.scalar.copy(out=wqT_h[:, :, 128 * hd : 128 * hd + 128], in_=trp[:])
                nc.scalar.copy(out=wq8[:, :, 1, 128 * hd : 128 * hd + 128], in_=trp[:])
                wlo = wtmp.tile([128, C], f16, tag="wlo")
                nc.vector.tensor_tensor(
                    out=wlo[:], in0=wnat[:], in1=whi[:], op=mybir.AluOpType.subtract
                )
                trp2 = pe_transpose_psum(wlo[:])
                nc.scalar.mul(wq8[:, :, 0, 128 * hd : 128 * hd + 128], trp2[:], SC)
            nc.vector.memset(wk_nat[96:128, :, :], 0.0)
            for hd in range(H):
                eng = nc.sync if hd % 2 == 0 else nc.scalar
                eng.dma_start(
                    out=wk_nat[0:HD, hd, :], in_=wk_d[HD * hd : HD * hd + HD, :]
                )

            # Wv/Wo prep split into per-block steps, spread across the chunk
            # loop (one DMA + one transpose per chunk, software-pipelined).
            wvwo_blocks = [(wv_d, wvT_h, t) for t in range(CT)] + [
                (wo_d, woT_h, t) for t in range(CT)
            ]
            wvwo_staged = []
            wv_step = [0]

            def prep_wv_wo_step(step):
                # stage the DMA for block `step`, transpose block `step-1`
                if step < len(wvwo_blocks):
                    w_dram, dst_h, t = wvwo_blocks[step]
                    wnat = wtmp.tile([128, C], f32, tag="wnat")
                    eng = nc.sync if step % 2 == 0 else nc.scalar
                    eng.dma_start(
                        out=wnat[:], in_=w_dram[128 * t : 128 * t + 128, :]
                    )
                    whi = wtmp.tile([128, C], f16, tag="whi")
                    nc.vector.tensor_copy(whi[:], wnat[:])
                    wvwo_staged.append((whi, dst_h, t))
                if step > 0 and step - 1 < len(wvwo_blocks):
                    whi, dst_h, t = wvwo_staged[step - 1]
                    pe_transpose_blocks(whi[:], dst_h[:], t)
                if step - 1 == len(wvwo_blocks) - 1:
                    wtmp_ctx.__exit__(None, None, None)

            qy_tiles = {}

            def prep_qy(b):
                # ---- Q path ----
                q_nat = qpool.tile([NG, C], f32, tag="qnat")
                nc.sync.dma_start(out=q_nat[:], in_=query_d[b])
                qh_nat = qpool.tile([NG, C], f16, tag="qhnat")
                ql_nat = qpool.tile([NG, C], f16, tag="qlnat")
                nc.vector.tensor_copy(qh_nat[:], q_nat[:])
                nc.vector.tensor_tensor(
                    out=ql_nat[:], in0=q_nat[:], in1=qh_nat[:], op=mybir.AluOpType.subtract
                )
                # queryT (c-major) fp16 halves via PE transpose ([64,128] blocks)
                qTq_h = qpool.tile([128, CT, NG], f16, tag="qTqh")
                qTq_l = qpool.tile([128, CT, NG], f16, tag="qTql")
                for qsrc, dst in ((qh_nat, qTq_h), (ql_nat, qTq_l)):
                    pe_transpose_blocks(qsrc, dst[:].unsqueeze(3).rearrange("p u n o -> p u (n o)"), 0, rows=NG)
                # DR operands for q-proj: qs16 = qh*2^12, q8 = [fp8(qh), fp8(ql*2^12)]
                qs16 = qpool.tile([128, CT, NG], f16, tag="qs16")
                q8 = qpool.tile([128, CT, 2, NG], f8, tag="q8")
                nc.scalar.mul(qs16[:], qTq_h[:], SC)
                nc.scalar.copy(out=q8[:, :, 0, :], in_=qTq_h[:])
                nc.scalar.mul(q8[:, :, 1, :], qTq_l[:], SC)
                # q projection (natural layout, M=64), d-padded: q_pad [64, 1024]
                # 6 fp16 + 6 DR matmuls at 2^12 scale, unscaled on the ACT copy
                q_sb = qpool.tile([NG, CP], f32, tag="qsb")
                for half in range(2):
                    nsl = slice(512 * half, 512 * half + 512)
                    nsl8 = slice(512 * half, 512 * half + 512)
                    qp = ps_a.tile([NG, 512], f32, tag="psa")
                    for u in range(CT):
                        nc.tensor.matmul(
                            qp[:],
                            qs16[:, u, :],
                            wqT_h[:, u, nsl],
                            start=(u == 0),
                            stop=False,
                        )
                    for u in range(CT):
                        nc.tensor.matmul(
                            qp[:],
                            q8[:, u, :, :],
                            wq8[:, u, :, nsl8],
                            start=False,
                            stop=(u == CT - 1),
                            perf_mode=mybir.MatmulPerfMode.DoubleRow,
                        )
                    nc.scalar.mul(q_sb[:, nsl], qp[:], 1.0 / SC)
                # qT (padded d-major, per head) fp32 via PE transpose
                qT = qpool.tile([128, H, NG], f32, tag="qT")
                for hd in range(H):
                    trq2 = ps_a.tile([128, NG], f32, tag="psa")
                    nc.tensor.matmul(
                        trq2[:],
                        q_sb[:, 128 * hd : 128 * hd + 128],
                        ident64_32[:],
                        is_transpose=True,
                        start=True,
                        stop=True,
                    )
                    nc.scalar.copy(out=qT[:, hd, :], in_=trq2[:])
                # Y_all[c, 64h+n] = sum_d Wk[d(head h), c] * q[n, d], fp32;
                # split into T1 rhs Yh16s = fp16(Y)*2^12 (exact p2 scale) and
                # fp8 planes y8 = [fp8(Yl*2^12), fp8(Yh16)].
                Yh16s = ypool.tile([128, CT, 8 * NG], f16, tag="Yh16s")
                y8 = ypool.tile([128, CT, 2, 8 * NG], f8, tag="y8")
                for u_c in range(CT):
                    yp = ps_a.tile([128, 8 * NG], f32, tag="psa")
                    csl = slice(128 * u_c, 128 * u_c + 128)
                    for hd in range(H):
                        nc.tensor.matmul(
                            yp[:, NG * hd : NG * hd + NG],
                            wk_nat[:, hd, csl],
                            qT[:, hd, :],
                            start=(hd == 0),
                            stop=(hd == H - 1),
                        )
                    yh = ytmp.tile([128, 8 * NG], f16, tag="yh")
                    nc.vector.tensor_copy(yh[:], yp[:])
                    nc.scalar.mul(Yh16s[:, u_c, :], yh[:], SC)
                    ylt = ytmp.tile([128, 8 * NG], f16, tag="ylt")
                    nc.vector.tensor_tensor(
                        out=ylt[:], in0=yp[:], in1=yh[:], op=mybir.AluOpType.subtract
                    )
                    nc.scalar.mul(y8[:, u_c, 0, :], ylt[:], SC)
                    nc.scalar.copy(out=y8[:, u_c, 1, :], in_=yh[:])
                qy_tiles[b] = (Yh16s, y8)

            for b in range(b_sh):
                if b not in qy_tiles:
                    prep_qy(b)
                Yh16s, y8 = qy_tiles.pop(b)

                # ---- raw-key group-sum accumulators (head-pair packed):
                # gsr[j][n(2 heads), c-half+count] = sum_s onehot[s, n] key[s, c]
                gsr = [ps_g4.tile([128, 509], f32, tag="g4", name=f"gsr{_j}") for _j in range(4)]
                oh_tiles = []
                gs_pending = None

                def emit_gs1(oh_t, khi_t, i_t, first_t, last_t):
                    for j in range(4):
                        nc.tensor.matmul(
                            gsr[j][:],
                            oh_t[:, 128 * j : 128 * j + 128],
                            khi_t[:, i_t, :],
                            start=first_t,
                            stop=last_t,
                        )

                for chunk in range(n_chunks):
                    if (b, chunk) not in key_tiles:
                        phase_k(b, chunk)
                    if b == 0 and chunk >= 3 and wv_step[0] <= len(wvwo_blocks):
                        prep_wv_wo_step(wv_step[0])
                        wv_step[0] += 1
                    if b == 0 and b_sh > 1 and chunk == 12:
                        prep_qy(1)
                    kTh, k8, khi = key_tiles.pop((b, chunk))

                    for i in range(n_sub):
                        ssl = slice(128 * i, 128 * i + 128)
                        # logits*2^12 for all 8 heads: 6 fp16 + 6 fp8-DR MMs
                        # into one PSUM accumulation group.
                        lg = ps_a.tile([128, 8 * NG], f32, tag="psa")
                        for u_c in range(CT):
                            nc.tensor.matmul(
                                lg[:],
                                kTh[:, u_c, ssl],
                                Yh16s[:, u_c, :],
                                start=(u_c == 0),
                                stop=False,
                            )
                        for u_c in range(CT):
                            nc.tensor.matmul(
                                lg[:],
                                k8[:, u_c, :, ssl],
                                y8[:, u_c, :, :],
                                start=False,
                                stop=(u_c == CT - 1),
                                perf_mode=mybir.MatmulPerfMode.DoubleRow,
                            )
                        # argmax -> one-hot via (x >= rowmax), fp16
                        mx = mxp.tile([128, H], f32, tag="mx")
                        lg3 = lg[:].rearrange("p (h n) -> p h n", h=H)
                        nc.vector.tensor_reduce(
                            out=mx[:],
                            in_=lg3,
                            axis=mybir.AxisListType.X,
                            op=mybir.AluOpType.max,
                        )
                        # one-hot in fp8 (0/1 exact; fp8-lhsT x f16-rhs
                        # matmul verified exact on HW) -- halves oh SBUF
                        oh = ohp.tile([128, H * NG], f8, tag="oh")
                        nc.vector.tensor_tensor(
                            out=oh[:].rearrange("p (h n) -> p h n", h=H),
                            in0=lg3,
                            in1=mx[:].unsqueeze(2).to_broadcast((128, H, NG)),
                            op=mybir.AluOpType.is_ge,
                        )

                        oh_tiles.append(oh)
                        # pass 1 gs_raw is emitted one subtile LATE (after the
                        # next subtile's logits) so the PE never waits on the
                        # DVE argmax: gs(i) lands in the PE queue after
                        # logits(i+1).
                        if gs_pending is not None:
                            emit_gs1(*gs_pending)
                        gs_pending = (oh, khi, i, chunk == 0 and i == 0, False)
                if gs_pending is not None:
                    # flush the final subtile's gs matmuls (accum group stop)
                    oh_p, khi_p, i_p, first_p, _ = gs_pending
                    emit_gs1(oh_p, khi_p, i_p, first_p, True)
                    gs_pending = None

                while b == 0 and wv_step[0] <= len(wvwo_blocks):
                    prep_wv_wo_step(wv_step[0])
                    wv_step[0] += 1

                # ---- recip of counts, divide pass-1 halves into ga ----
                cnts = outp.tile([128, 4], f32, tag="cnts")
                recs = outp.tile([128, 4], f32, tag="recs")
                ga = outp.tile([128, 4, C], f16, tag="ga")
                for j in range(4):
                    nc.vector.tensor_scalar(
                        out=cnts[:, j : j + 1], in0=gsr[j][:, 508:509],
                        scalar1=1.0, scalar2=None, op0=mybir.AluOpType.add,
                    )
                    nc.vector.reciprocal(recs[:, j : j + 1], cnts[:, j : j + 1])
                    nc.vector.tensor_scalar(
                        out=ga[:, j, 0:508], in0=gsr[j][:, 0:508],
                        scalar1=recs[:, j : j + 1], scalar2=None,
                        op0=mybir.AluOpType.mult,
                    )

                # ---- vproj part A (c 0:384) overlaps pass 2: transpose the
                # ready ga columns and accumulate the first 3 c-chunks of the
                # Wv projection while pass 2 streams.
                gaT = outp.tile([128, CT, 4, 128], f16, tag="gaT")
                for j in range(4):
                    pe_transpose_blocks(ga[:, j, :], gaT[:, :, j, :], 0, u0=0, u1=3)
                attn16 = outp.tile([NG, C], f16, tag="attn16")
                for h in range(H):
                    pa = ps_a.tile([NG, HD], f32, tag="psa")
                    for u_c in range(3):
                        nc.tensor.matmul(
                            pa[:],
                            gaT[:, u_c, h // 2, 64 * (h % 2) : 64 * (h % 2) + 64],
                            wvT_h[:, u_c, HD * h : HD * h + HD],
                            start=(u_c == 0),
                            stop=(u_c == 2),
                        )
                    nc.scalar.copy(out=attn16[:, HD * h : HD * h + HD], in_=pa[:])

                # ---- pass 2: re-stream key, gs_raw over c[508:768] ----
                gsr2 = [ps_g4.tile([128, 260], f32, tag="g4", name=f"gsr2_{_j}") for _j in range(4)]
                for chunk in range(n_chunks):
                    s0 = chunk * S_CHUNK
                    knat2 = kin.tile([128, n_sub, C], f32, tag="knat")
                    nc.gpsimd.dma_start(
                        out=knat2[:, :, 0:260],
                        in_=key_d[b, s0 : s0 + S_CHUNK, 508:768].rearrange(
                            "(i p) c -> p i c", p=128
                        ),
                    )
                    khi2 = khip.tile([128, n_sub, 509], f16, tag="khi")
                    nc.vector.tensor_copy(khi2[:, :, 0:260], knat2[:, :, 0:260])
                    for i in range(n_sub):
                        last = chunk == n_chunks - 1 and i == n_sub - 1
                        first = chunk == 0 and i == 0
                        oh_t = oh_tiles[chunk * n_sub + i]
                        for j in range(4):
                            nc.tensor.matmul(
                                gsr2[j][:, 0:260],
                                oh_t[:, 128 * j : 128 * j + 128],
                                khi2[:, i, 0:260],
                                start=first,
                                stop=last,
                            )
                    # overlap batch 1's first key preps with batch 0 pass 2
                    if b == 0 and b_sh > 1 and chunk == n_chunks - 2:
                        phase_k(1, 0)
                    if b == 0 and b_sh > 1 and chunk == n_chunks - 1:
                        phase_k(1, 1)
                for j in range(4):
                    nc.vector.tensor_scalar(
                        out=ga[:, j, 508:768], in0=gsr2[j][:, 0:260],
                        scalar1=recs[:, j : j + 1], scalar2=None,
                        op0=mybir.AluOpType.mult,
                    )

                # ---- vproj part B (c 384:768): transpose remaining ga
                # columns, accumulate, and DVE-add into attn16 ----
                for j in range(4):
                    pe_transpose_blocks(ga[:, j, :], gaT[:, :, j, :], 0, u0=3, u1=CT)
                for h in range(H):
                    pa = ps_a.tile([NG, HD], f32, tag="psa")
                    for u_c in range(3, CT):
                        nc.tensor.matmul(
                            pa[:],
                            gaT[:, u_c, h // 2, 64 * (h % 2) : 64 * (h % 2) + 64],
                            wvT_h[:, u_c, HD * h : HD * h + HD],
                            start=(u_c == 3),
                            stop=(u_c == CT - 1),
                        )
                    nc.vector.tensor_tensor(
                        out=attn16[:, HD * h : HD * h + HD],
                        in0=pa[:],
                        in1=attn16[:, HD * h : HD * h + HD],
                        op=mybir.AluOpType.add,
                    )
                attnT = outp.tile([128, CT, NG], f16, tag="attnT")
                pe_transpose_blocks(attn16, attnT[:].unsqueeze(3).rearrange("p u n o -> p u (n o)"), 0, rows=NG)

                out_sb = outp.tile([NG, C], f32, tag="outsb")
                for half in range(2):
                    nsl = slice(384 * half, 384 * half + 384)
                    op = ps_a.tile([NG, 384], f32, tag="psa")
                    for u_c in range(CT):
                        nc.tensor.matmul(
                            op[:],
                            attnT[:, u_c, :],
                            woT_h[:, u_c, nsl],
                            start=(u_c == 0),
                            stop=False,
                        )
                    nc.tensor.matmul(
                        op[:], ones_row[:], bo_sb[:, nsl], start=False, stop=True
                    )
                    nc.scalar.copy(out=out_sb[:, nsl], in_=op[:])
                nc.gpsimd.dma_start(out=out_d[b], in_=out_sb[:])

    nc.finalize()
    return nc


_NC_CACHE = {}


def _get_nc(b_sh, S):
    key = (b_sh, S)
    if key not in _NC_CACHE:
        _NC_CACHE[key] = build_nc(b_sh, S)
    return _NC_CACHE[key]


def kernel(query, key_in, Wq, Wk, Wv, Wo, bo):
    from concourse.bass_utils import run_bass_kernel_spmd

    query = np.ascontiguousarray(np.asarray(query, dtype=np.float32))
    key_in = np.ascontiguousarray(np.asarray(key_in, dtype=np.float32))
    Wq = np.ascontiguousarray(np.asarray(Wq, dtype=np.float32))
    Wk = np.ascontiguousarray(np.asarray(Wk, dtype=np.float32))
    Wv = np.ascontiguousarray(np.asarray(Wv, dtype=np.float32))
    Wo = np.ascontiguousarray(np.asarray(Wo, dtype=np.float32))
    bo = np.ascontiguousarray(np.asarray(bo, dtype=np.float32))

    B, _, _ = query.shape
    S = key_in.shape[1]
    n_cores = 8
    b_sh = B // n_cores
    nc = _get_nc(b_sh, S)

    in_maps = []
    for i in range(n_cores):
        bs = slice(i * b_sh, (i + 1) * b_sh)
        in_maps.append(
            {
                "query": np.ascontiguousarray(query[bs]),
                "key_in": np.ascontiguousarray(key_in[bs]),
                "Wq": Wq,
                "Wk": Wk,
                "Wv": Wv,
                "Wo": Wo,
                "bo": bo,
            }
        )
    res = run_bass_kernel_spmd(nc, in_maps, core_ids=list(range(n_cores)))
    out = np.concatenate([res.results[i]["out"] for i in range(n_cores)], axis=0)
    return out.astype(np.float32)


if __name__ == "__main__":
    nc = build_nc(1, 512)
    print("built ok")
